# revision 1
# baseline (speedup 1.0000x reference)
"""3-layer GAT (4 heads) on 8 trn2 NeuronCores.

Strategy: shard nodes by destination across 8 cores. Per layer:
  dense:  h = X @ [W | w_src | w_dst] per node shard (PE), AllGather the
          per-node gather table (bf16 h rows + fp32 attention logits).
  edge:   per 128-edge sub-chunk (edges pre-sorted by dst block on host):
          dma_gather source rows + dst logits, exp(leaky(as+ad)) on ACT,
          messages G = ex*h on DVE, aggregation + softmax denominator via
          one fp8 one-hot matmul per sub-chunk accumulating in PSUM per
          dst block.  Finalize: divide by denominator, +bias, relu,
          PE-transpose into next layer's input.
Everything (edge sorting, one-hot S matrices, gather index streams, fused
weight matrices) is precomputed on host in numpy.
"""
import numpy as np
import ml_dtypes

import concourse.bass as bass
import concourse.bacc as bacc
import concourse.tile as tile
from concourse import mybir
from concourse.bass_utils import run_bass_kernel_spmd

N = 50000
NC = 8
SH = 6250            # nodes per core
SHP = 6272           # padded (49 * 128)
NBLK = SHP // 128    # 49
HALF = 4 * SHP       # 25088: table row where half B starts
HEADS = 4
C = 64
HID = 256
F1 = 128
NEG = 0.2
CALL_K = 8           # max 128-edge sub-chunks per gather call

BF16 = mybir.dt.bfloat16
F32 = mybir.dt.float32
FP8 = mybir.dt.float8e4
I16 = mybir.dt.int16
NP_F8 = mybir.dt.np(FP8)
NP_BF16 = mybir.dt.np(BF16)

TBL = 384            # bf16 cols/row: 256 h | 8 (=4 fp32 alpha_src) | pad
ADC = 64             # f32 cols/row of the alpha_dst table (256B rows)


# ----------------------------------------------------------------- host prep
def _wvec(W, a):
    """w[k,h] = sum_c W[k, h*C+c] * a[h,c]  (alpha = (x@W).reshape(N,H,C)·a)"""
    fin = W.shape[0]
    return np.einsum("khc,hc->kh", W.reshape(fin, HEADS, C), a).astype(np.float32)


def _prep(x, edge_index, Ws, a_srcs, a_dsts, bs):
    E = edge_index.shape[1]
    loops = np.arange(N, dtype=np.int64)
    src = np.concatenate([np.asarray(edge_index[0], dtype=np.int64), loops])
    dst = np.concatenate([np.asarray(edge_index[1], dtype=np.int64), loops])
    core_of = dst // SH

    # ---- pass 1: per-core node->block permutation balancing per-half load
    src_half = (src // SH >= NC // 2).astype(np.int64)
    permpos = np.zeros(N, np.int64)   # node -> row within its core's shard
    for c in range(NC):
        m = core_of == c
        dl = (dst[m] - c * SH).astype(np.int64)
        sh_e = src_half[m]
        a_cnt = np.bincount(dl[sh_e == 0], minlength=SH)
        b_cnt = np.bincount(dl[sh_e == 1], minlength=SH)
        capA = 128.0 * -(-int(a_cnt.sum()) // (128 * NBLK))
        capB = 128.0 * -(-int(b_cnt.sum()) // (128 * NBLK))
        order = np.argsort(-(a_cnt + b_cnt), kind="stable")
        la = np.zeros(NBLK)
        lb = np.zeros(NBLK)
        slots = np.zeros(NBLK, np.int64)
        pos = np.zeros(SH, np.int64)
        for nid in order:
            na = la + a_cnt[nid]
            nb = lb + b_cnt[nid]
            score = (np.maximum(na / capA, nb / capB)
                     + 1e6 * ((na > capA) + (nb > capB)))
            score[slots >= 128] = 1e18
            j = int(np.argmin(score))
            pos[nid] = j * 128 + slots[j]
            slots[j] += 1
            la[j] += a_cnt[nid]
            lb[j] += b_cnt[nid]
        # refinement: swap nodes out of over-cap groups into slack groups
        blk_of = pos >> 7
        for _ in range(4000):
            exA = la - capA
            exB = lb - capB
            if max(exA.max(), exB.max()) <= 0:
                break
            if exA.max() >= exB.max():
                cnt, lo_, cap_, oth, lo2_, cap2_ = a_cnt, la, capA, b_cnt, lb, capB
            else:
                cnt, lo_, cap_, oth, lo2_, cap2_ = b_cnt, lb, capB, a_cnt, la, capA
            j1 = int(np.argmax(lo_))
            j2 = int(np.argmin(lo_))
            n1s = np.nonzero(blk_of == j1)[0]
            n2s = np.nonzero(blk_of == j2)[0]
            need = lo_[j1] - cap_
            best = None
            for n1 in n1s[np.argsort(-cnt[n1s])[:24]]:
                for n2 in n2s[np.argsort(cnt[n2s])[:24]]:
                    t = cnt[n1] - cnt[n2]
                    if t <= 0:
                        continue
                    to = oth[n1] - oth[n2]
                    if (lo_[j2] + t <= cap_ and lo2_[j2] + to <= cap2_
                            and lo2_[j1] - to <= cap2_):
                        sc = min(t, need)
                        if best is None or sc > best[0]:
                            best = (sc, n1, n2)
            if best is None:
                break
            _, n1, n2 = best
            p1, p2 = pos[n1], pos[n2]
            pos[n1], pos[n2] = p2, p1
            blk_of[n1], blk_of[n2] = j2, j1
            la[j1] += a_cnt[n2] - a_cnt[n1]
            la[j2] += a_cnt[n1] - a_cnt[n2]
            lb[j1] += b_cnt[n2] - b_cnt[n1]
            lb[j2] += b_cnt[n1] - b_cnt[n2]
        permpos[c * SH:(c + 1) * SH] = pos

    per_core = []
    counts = np.zeros((NC, NBLK, 2), np.int64)
    for c in range(NC):
        m = core_of == c
        s_c = src[m]
        p = permpos[dst[m]]
        blk = p >> 7
        rel = p & 127
        trow = (s_c // SH) * SHP + permpos[s_c]
        half = (trow >= HALF).astype(np.int64)
        lidx = trow - half * HALF
        o = np.lexsort((lidx, half, blk))
        blk, rel, half, lidx = blk[o], rel[o], half[o], lidx[o]
        gid = blk * 2 + half
        counts[c] = np.bincount(gid, minlength=NBLK * 2).reshape(NBLK, 2)
        per_core.append((blk, rel, half, lidx))
    assert counts.sum() == E + N

    K = -(-counts.max(axis=0) // 128)  # [NBLK, 2] sub-chunks per group
    # schedule: calls (half, k, first_sub); subs carry (blk, start, stop)
    calls = []          # (half, k, sub0)
    sub_blk = []
    sub_base = np.zeros((NBLK, 2), np.int64)
    for b in range(NBLK):
        for h in (0, 1):
            sub_base[b, h] = len(sub_blk)
            kk = int(K[b, h])
            while kk > 0:
                take = min(kk, CALL_K)
                calls.append((h, take, len(sub_blk)))
                sub_blk.extend([b] * take)
                kk -= take
    NSUB = len(sub_blk)
    NCALLS = len(calls)
    sub_blk = np.asarray(sub_blk)
    sub_start = np.zeros(NSUB, bool)
    sub_stop = np.zeros(NSUB, bool)
    for b in range(NBLK):
        w = np.nonzero(sub_blk == b)[0]
        assert len(w) > 0
        sub_start[w[0]] = True
        sub_stop[w[-1]] = True

    in_maps = []
    for c in range(NC):
        blk, rel, half, lidx = per_core[c]
        n = len(blk)
        gid = blk * 2 + half
        # position of each edge within its (blk,half) group
        grp_cnt = counts[c].reshape(-1)
        grp_off = np.concatenate([[0], np.cumsum(grp_cnt)])[:-1]
        pos = np.arange(n) - grp_off[gid]
        su = sub_base[blk, half] + (pos >> 7)
        epos = pos & 127

        midx = np.zeros((NSUB, 128), np.int16)
        midx[su, epos] = lidx.astype(np.int16)
        # interleaved per-sub: cols [0:128) = S^T[e,d]; [128:256) = S_dmaj[d,e]
        sst = np.zeros((128, NSUB * 256), NP_F8)
        sst[epos, su * 256 + rel] = 1.0
        sst[rel, su * 256 + 128 + epos] = 1.0

        def wrap(arr_subs):
            cw = CALL_K * 8
            o = np.zeros((128, NCALLS * cw), np.int16)
            for ci, (h, k, s0) in enumerate(calls):
                flat = arr_subs[s0:s0 + k].reshape(k * 128)
                w16 = flat.reshape(k * 8, 16).T  # [16, k*8]
                o[:, ci * cw:ci * cw + k * 8] = np.tile(w16, (8, 1))
            return o

        xs = x[c * SH:(c + 1) * SH].astype(np.float32)
        xT = np.zeros((F1, SHP), np.float32)
        xT[:, permpos[c * SH:(c + 1) * SH]] = xs.T

        m = {
            "xT": xT,
            "idxm": wrap(midx),
            "sst": sst,
            "ident": np.eye(128, dtype=np.float32),
        }
        for li in range(3):
            W = np.asarray(Ws[li], np.float32)
            wf = np.concatenate(
                [W, _wvec(W, np.asarray(a_srcs[li], np.float32)),
                 _wvec(W, np.asarray(a_dsts[li], np.float32))], axis=1)  # [fin, 264]
            fin = wf.shape[0]
            kt = fin // 128
            m[f"wf{li + 1}"] = wf.reshape(kt, 128, 264).transpose(1, 0, 2).copy()
            m[f"b{li + 1}"] = np.tile(np.asarray(bs[li], np.float32)[None, :], (128, 1))
        in_maps.append(m)

    sched = dict(calls=calls, NSUB=NSUB, NCALLS=NCALLS,
                 sub_blk=sub_blk, sub_start=sub_start, sub_stop=sub_stop,
                 permpos=permpos)
    return in_maps, sched


# --------------------------------------------------------------- bass kernel
def _build(sched, profile=False):
    calls = sched["calls"]
    NSUB, NCALLS = sched["NSUB"], sched["NCALLS"]
    sub_blk, sub_start, sub_stop = (
        sched["sub_blk"], sched["sub_start"], sched["sub_stop"])

    nc = bacc.Bacc("TRN2", target_bir_lowering=False, debug=False,
                   num_devices=(1 if profile else NC),
                   num_swdge_queues=4, dynamic_dma_scratch_size=65536)

    xT_d = nc.dram_tensor("xT", [F1, SHP], F32, kind="ExternalInput")
    idxm_d = nc.dram_tensor("idxm", [128, NCALLS * CALL_K * 8], I16, kind="ExternalInput")
    sst_d = nc.dram_tensor("sst", [128, NSUB * 256], FP8, kind="ExternalInput")
    ident_d = nc.dram_tensor("ident", [128, 128], F32, kind="ExternalInput")
    wf_d = [nc.dram_tensor(f"wf{li + 1}", [128, (1 if li == 0 else 2), 264], F32,
                           kind="ExternalInput") for li in range(3)]
    b_d = [nc.dram_tensor(f"b{li + 1}", [128, 256], F32, kind="ExternalInput")
           for li in range(3)]
    out_d = nc.dram_tensor("out", [SHP, 256], F32, kind="ExternalOutput")

    with tile.TileContext(nc) as tc:
        with (
            tc.tile_pool(name="const", bufs=1) as const,
            tc.tile_pool(name="xt", bufs=1) as xtp,
            tc.tile_pool(name="tp", bufs=5) as tp,
            tc.tile_pool(name="sp", bufs=5) as sp,
            tc.tile_pool(name="gp", bufs=3) as gp,
            tc.tile_pool(name="ep", bufs=4) as ep,
            tc.tile_pool(name="fp", bufs=2) as fpool,
            tc.tile_pool(name="dn", bufs=2) as dn,
            tc.tile_pool(name="psb", bufs=3, space="PSUM") as psb,
            tc.tile_pool(name="psd", bufs=1, space="PSUM") as psd,
            tc.tile_pool(name="pst", bufs=2, space="PSUM") as pst,
            tc.tile_pool(name="pse", bufs=2, space="PSUM") as pse,
            tc.tile_pool(name="dram", bufs=1, space="DRAM") as dram,
        ):
            ident = const.tile([128, 128], F32, tag="ident")
            nc.sync.dma_start(ident[:], ident_d[:])
            idxm = const.tile([128, NCALLS * CALL_K * 8], I16, tag="idxm")
            nc.sync.dma_start(idxm[:], idxm_d[:])
            adall = const.tile([128, NBLK, 8], BF16, tag="adall")
            wf_sb = []
            b_sb = []
            for li in range(3):
                kt = 1 if li == 0 else 2
                w = const.tile([128, kt, 264], F32, tag=f"wf{li}")
                nc.sync.dma_start(w[:], wf_d[li][:])
                wf_sb.append(w)
                bt = const.tile([128, 256], F32, tag=f"b{li}")
                nc.sync.dma_start(bt[:], b_d[li][:])
                b_sb.append(bt)

            XT = []
            for b in range(NBLK):
                xtb = xtp.tile([128, 2, 128], F32, tag=f"XT{b}")
                XT.append(xtb)
                nc.sync.dma_start(xtb[:, 0, :], xT_d[:, b * 128:(b + 1) * 128])

            tables = []
            for li in range(3):
                t_sh = dram.tile([SHP, TBL], BF16, tag=f"tsh{li}")
                t_full = dram.tile([NC * SHP, TBL], BF16, tag=f"tfull{li}",
                                   addr_space="Shared")
                tables.append((t_sh, t_full))

            for li in range(3):
                kt = 1 if li == 0 else 2
                t_sh, t_full = tables[li]
                # ---------------- dense: h | as | ad = X @ wf ----------------
                for t in range(NBLK):
                    pd = psd.tile([128, 264], F32, tag="pd")
                    for kk in range(kt):
                        nc.tensor.matmul(
                            pd[:], XT[t][:, kk, :],
                            wf_sb[li][:, kk, :],
                            start=(kk == 0), stop=(kk == kt - 1))
                    tb = dn.tile([128, TBL], BF16, tag="tb")
                    nc.vector.tensor_copy(tb[:, 0:256], pd[:, 0:256])
                    tbf = tb[:].bitcast(F32)  # [128, 192]
                    nc.vector.tensor_copy(tbf[:, 128:132], pd[:, 256:260])
                    nc.sync.dma_start(t_sh[t * 128:(t + 1) * 128, :], tb[:])
                    nc.vector.tensor_copy(adall[:, t, 0:4], pd[:, 260:264])
                    adr = dn.tile([128, 4], F32, tag="adr")
                    nc.vector.tensor_tensor(
                        adr[:], pd[:, 260:264], adall[:, t, 0:4],
                        mybir.AluOpType.subtract)
                    nc.vector.tensor_copy(adall[:, t, 4:8], adr[:])
                # ---------------- all-gather the table ----------------------
                if profile:
                    nc.sync.dma_start(t_full[0:SHP, :], t_sh[:])
                else:
                    nc.gpsimd.collective_compute(
                        "AllGather", mybir.AluOpType.bypass,
                        ins=[t_sh[:]], outs=[t_full[:]],
                        replica_groups=[list(range(NC))])
                # ---------------- edge phase (sw-pipelined emission) ---------
                pcur = None
                live = {}

                def emit_gather(ci):
                    h, k, s0 = calls[ci]
                    tt = tp.tile([128, CALL_K, TBL], BF16, tag="tt")
                    nc.gpsimd.dma_gather(
                        tt[:, :k, :], t_full[h * HALF:(h + 1) * HALF, :],
                        idxm[:, ci * CALL_K * 8:ci * CALL_K * 8 + k * 8],
                        k * 128, k * 128, TBL, queue_num=ci % 4)
                    st = sp.tile([128, CALL_K * 256], FP8, tag="st")
                    nc.scalar.dma_start(
                        st[:, :k * 256], sst_d[:, s0 * 256:(s0 + k) * 256])
                    live[ci] = [tt, st]

                def emit_score(ci):
                    h, k, s0 = calls[ci]
                    tt, st = live[ci]
                    pe_ps = pse.tile([128, CALL_K * 4], F32, tag="pe")
                    for s in range(k):
                        b = int(sub_blk[s0 + s])
                        sd = st[:, s * 256 + 128:s * 256 + 256]
                        nc.tensor.matmul(
                            pe_ps[:, s * 4:(s + 1) * 4], sd,
                            adall[:, b, 0:4], start=True, stop=False)
                        nc.tensor.matmul(
                            pe_ps[:, s * 4:(s + 1) * 4], sd,
                            adall[:, b, 4:8], start=False, stop=True)
                    epre = ep.tile([128, CALL_K, 4], F32, tag="epre")
                    nc.vector.tensor_tensor(
                        epre[:, :k, :], tt[:].bitcast(F32)[:, :k, 128:132],
                        pe_ps[:].rearrange("p (s f) -> p s f", f=4)[:, :k, :],
                        mybir.AluOpType.add)
                    live[ci].append(pe_ps)
                    elr = ep.tile([128, CALL_K, 4], F32, tag="elr")
                    nc.vector.scalar_tensor_tensor(
                        elr[:, :k, :], epre[:, :k, :], NEG, epre[:, :k, :],
                        mybir.AluOpType.mult, mybir.AluOpType.max)
                    g = gp.tile([128, CALL_K, 260], BF16, tag="g")
                    g4 = g[:, :k, :].rearrange("p s (h c) -> p s h c", h=HEADS)
                    nc.scalar.activation(
                        g4[:, :, :, 64], elr[:, :k, :],
                        mybir.ActivationFunctionType.Exp)
                    live[ci].append(g)



                def emit_agg(ci):
                    nonlocal pcur
                    h, k, s0 = calls[ci]
                    tt, st, pe_ps, g = live.pop(ci)
                    g4 = g[:, :k, :].rearrange("p s (h c) -> p s h c", h=HEADS)
                    exb = g4[:, :, :, 64].unsqueeze(3).broadcast_to(
                        [128, k, HEADS, C])
                    nc.vector.tensor_tensor(
                        g4[:, :, :, 0:64],
                        tt[:, :k, 0:256].rearrange("p s (h c) -> p s h c", h=HEADS),
                        exb, mybir.AluOpType.mult)
                    for s in range(k):
                        su = s0 + s
                        if sub_start[su]:
                            pcur = psb.tile([128, 260], F32, tag="pb")
                        nc.tensor.matmul(
                            pcur[:], st[:, s * 256:s * 256 + 128], g[:, s, :],
                            start=bool(sub_start[su]), stop=bool(sub_stop[su]))
                        if sub_stop[su]:
                            b = int(sub_blk[su])
                            ph = pcur[:].rearrange("p (h c) -> p h c", h=HEADS)
                            rec = fpool.tile([128, 4], F32, tag="rec")
                            nc.vector.reciprocal(rec[:], ph[:, :, 64])
                            fin = fpool.tile([128, 256], F32, tag="fin")
                            for hh in range(HEADS):
                                nc.vector.scalar_tensor_tensor(
                                    fin[:, hh * 64:(hh + 1) * 64],
                                    pcur[:, hh * 65:hh * 65 + 64],
                                    rec[:, hh:hh + 1],
                                    b_sb[li][:, hh * 64:(hh + 1) * 64],
                                    mybir.AluOpType.mult, mybir.AluOpType.add)
                            if li < 2:
                                xr = fpool.tile([128, 256], F32, tag="xr")
                                nc.vector.tensor_scalar_max(xr[:], fin[:], 0.0)
                                for half in range(2):
                                    pt = pst.tile([128, 128], F32, tag="pt")
                                    nc.tensor.transpose(
                                        pt[:], xr[:, half * 128:(half + 1) * 128],
                                        ident[:])
                                    nc.vector.tensor_copy(
                                        XT[b][:, half, :], pt[:])
                            else:
                                nc.sync.dma_start(
                                    out_d[b * 128:(b + 1) * 128, :], fin[:])

                for ci in range(len(calls) + 3):
                    if ci < len(calls):
                        emit_gather(ci)
                    if ci >= 2 and ci - 2 < len(calls):
                        emit_score(ci - 2)
                    if ci >= 3:
                        emit_agg(ci - 3)
    nc.compile()
    return nc


_CACHE = {}


def _get(x, edge_index, Ws, a_srcs, a_dsts, bs):
    in_maps, sched = _prep(x, edge_index, Ws, a_srcs, a_dsts, bs)
    nc = _build(sched)
    return nc, in_maps, sched


def kernel(x, edge_index, W1, a_src1, a_dst1, b1, W2, a_src2, a_dst2, b2,
           W3, a_src3, a_dst3, b3):
    x = np.asarray(x)
    nc, in_maps, sched = _get(x, np.asarray(edge_index),
                              [W1, W2, W3], [a_src1, a_src2, a_src3],
                              [a_dst1, a_dst2, a_dst3], [b1, b2, b3])
    res = run_bass_kernel_spmd(nc, in_maps, core_ids=list(range(NC)))
    permpos = sched["permpos"]
    out = np.concatenate(
        [res.results[c]["out"][permpos[c * SH:(c + 1) * SH]] for c in range(NC)],
        axis=0)
    return out[:N].astype(np.float32)



# revision 4
# speedup vs baseline: 1.5551x; 1.5551x over previous
"""3-layer GAT (4 heads) on 8 trn2 NeuronCores — v2.

Strategy: shard nodes by destination across 8 cores. Per layer:
  dense:  pd = X @ [W | w_src | w_dst] per node shard (PE, bf16).
          Table row (512B fp8): h fp8 (256B) | E1=exp(as), E2=exp(.2 as)
          as 8 bf16 (16B) | pad.  Locally keep F1=exp(ad), F2=exp(.2 ad)
          (fdata) and ex_self = max(E1 F1, E2 F2) per node.
          AllGather the fp8 table (3.2MB/core).
  edge:   exp(leaky(as+ad)) == max(exp(as)exp(ad), exp(.2as)exp(.2ad)),
          so per edge only the gathered E-pairs and one one-hot matmul
          for the dst F-pairs are needed (no per-edge ACT exp).
          Self-loop edges are NOT gathered: their contribution is added
          at finalize from t_sh rows + ex_self.
          Sub-chunks (128 edges, one dst block, one src half) are packed
          into gather calls of up to 8 subs spanning dst blocks (zigzag
          half order) to amortize the ~0.9us fixed cost per dma_gather.
Host precomputes edge sorting, one-hot S matrices, gather index streams,
fused bf16 weights.
"""
import numpy as np
import ml_dtypes

import concourse.bass as bass
import concourse.bacc as bacc
import concourse.tile as tile
from concourse import mybir
from concourse.bass_utils import run_bass_kernel_spmd

N = 50000
NC = 8
SH = 6250            # nodes per core
SHP = 6272           # padded (49 * 128)
NBLK = SHP // 128    # 49
HALF = 4 * SHP       # 25088: table row where half B starts
HEADS = 4
C = 64
NEG = 0.2
CALL_K = 8           # max 128-edge sub-chunks per gather call (ucode limit)
TBLB = 512           # fp8 cols (bytes) per table row

BF16 = mybir.dt.bfloat16
F32 = mybir.dt.float32
FP8 = mybir.dt.float8e4
I16 = mybir.dt.int16
NP_F8 = mybir.dt.np(FP8)
NP_BF16 = mybir.dt.np(BF16)


# ----------------------------------------------------------------- host prep
def _wvec(W, a):
    """w[k,h] = sum_c W[k, h*C+c] * a[h,c]  (alpha = (x@W).reshape(N,H,C)·a)"""
    fin = W.shape[0]
    return np.einsum("khc,hc->kh", W.reshape(fin, HEADS, C), a).astype(np.float32)


def _balance(src, dst, core_of):
    """Per-core node->row permutation balancing per-half gather load."""
    src_half = (src // SH >= NC // 2).astype(np.int64)
    permpos = np.zeros(N, np.int64)
    for c in range(NC):
        m = core_of == c
        dl = (dst[m] - c * SH).astype(np.int64)
        sh_e = src_half[m]
        a_cnt = np.bincount(dl[sh_e == 0], minlength=SH)
        b_cnt = np.bincount(dl[sh_e == 1], minlength=SH)
        capA = 128.0 * -(-int(a_cnt.sum()) // (128 * NBLK))
        capB = 128.0 * -(-int(b_cnt.sum()) // (128 * NBLK))
        order = np.argsort(-(a_cnt + b_cnt), kind="stable")
        la = np.zeros(NBLK)
        lb = np.zeros(NBLK)
        slots = np.zeros(NBLK, np.int64)
        pos = np.zeros(SH, np.int64)
        for nid in order:
            na = la + a_cnt[nid]
            nb = lb + b_cnt[nid]
            score = (np.maximum(na / capA, nb / capB)
                     + 1e6 * ((na > capA) + (nb > capB)))
            score[slots >= 128] = 1e18
            j = int(np.argmin(score))
            pos[nid] = j * 128 + slots[j]
            slots[j] += 1
            la[j] += a_cnt[nid]
            lb[j] += b_cnt[nid]
        blk_of = pos >> 7
        for _ in range(4000):
            exA = la - capA
            exB = lb - capB
            if max(exA.max(), exB.max()) <= 0:
                break
            if exA.max() >= exB.max():
                cnt, lo_, cap_, oth, lo2_, cap2_ = a_cnt, la, capA, b_cnt, lb, capB
            else:
                cnt, lo_, cap_, oth, lo2_, cap2_ = b_cnt, lb, capB, a_cnt, la, capA
            j1 = int(np.argmax(lo_))
            j2 = int(np.argmin(lo_))
            n1s = np.nonzero(blk_of == j1)[0]
            n2s = np.nonzero(blk_of == j2)[0]
            need = lo_[j1] - cap_
            best = None
            for n1 in n1s[np.argsort(-cnt[n1s])[:24]]:
                for n2 in n2s[np.argsort(cnt[n2s])[:24]]:
                    t = cnt[n1] - cnt[n2]
                    if t <= 0:
                        continue
                    to = oth[n1] - oth[n2]
                    if (lo_[j2] + t <= cap_ and lo2_[j2] + to <= cap2_
                            and lo2_[j1] - to <= cap2_):
                        sc = min(t, need)
                        if best is None or sc > best[0]:
                            best = (sc, n1, n2)
            if best is None:
                break
            _, n1, n2 = best
            p1, p2 = pos[n1], pos[n2]
            pos[n1], pos[n2] = p2, p1
            blk_of[n1], blk_of[n2] = j2, j1
            la[j1] += a_cnt[n2] - a_cnt[n1]
            la[j2] += a_cnt[n1] - a_cnt[n2]
            lb[j1] += b_cnt[n2] - b_cnt[n1]
            lb[j2] += b_cnt[n1] - b_cnt[n2]
        permpos[c * SH:(c + 1) * SH] = pos
    return permpos


def _prep(x, edge_index, Ws, a_srcs, a_dsts, bs):
    E = edge_index.shape[1]
    src = np.asarray(edge_index[0], dtype=np.int64)
    dst = np.asarray(edge_index[1], dtype=np.int64)
    core_of = dst // SH

    permpos = _balance(src, dst, core_of)

    per_core = []
    counts = np.zeros((NC, NBLK, 2), np.int64)
    for c in range(NC):
        m = core_of == c
        s_c = src[m]
        p = permpos[dst[m]]
        blk = p >> 7
        rel = p & 127
        trow = (s_c // SH) * SHP + permpos[s_c]
        half = (trow >= HALF).astype(np.int64)
        lidx = trow - half * HALF
        counts[c] = np.stack([
            np.bincount(blk[half == 0], minlength=NBLK),
            np.bincount(blk[half == 1], minlength=NBLK)], axis=1)
        per_core.append((blk, rel, half, lidx))
    assert counts.sum() == E

    K = -(-counts.max(axis=0) // 128)  # [NBLK, 2] sub-chunks per group
    # zigzag stream of (blk, half) groups so same-half runs merge across
    # adjacent blocks; calls pack <=8 consecutive same-half subs.
    stream = []
    for b in range(NBLK):
        stream += [(b, 0), (b, 1)] if b % 2 == 0 else [(b, 1), (b, 0)]
    sub_blk = []
    sub_half = []
    sub_base = {}
    gorder = {}
    for gi, (b, h) in enumerate(stream):
        gorder[(b, h)] = gi
        sub_base[(b, h)] = len(sub_blk)
        kk = int(K[b, h])
        assert kk >= 1
        sub_blk += [b] * kk
        sub_half += [h] * kk
    NSUB = len(sub_blk)
    sub_blk = np.asarray(sub_blk)
    sub_half = np.asarray(sub_half)

    calls = []          # (half, k, sub0)
    i = 0
    while i < NSUB:
        h = sub_half[i]
        j = i
        while j < NSUB and sub_half[j] == h and j - i < CALL_K:
            j += 1
        calls.append((int(h), j - i, i))
        i = j
    NCALLS = len(calls)
    call_off = np.zeros(NCALLS, np.int64)
    off = 0
    for ci, (h, k, s0) in enumerate(calls):
        call_off[ci] = off
        off += k * 8
    IDXC = int(off)

    sub_start = np.zeros(NSUB, bool)
    sub_stop = np.zeros(NSUB, bool)
    sub_first = np.zeros(NSUB, bool)   # first sub of its block (prefetch)
    for b in range(NBLK):
        w = np.nonzero(sub_blk == b)[0]
        assert len(w) > 0 and (np.diff(w) == 1).all()
        sub_start[w[0]] = True
        sub_stop[w[-1]] = True
        sub_first[w[0]] = True

    in_maps = []
    for c in range(NC):
        blk, rel, half, lidx = per_core[c]
        n = len(blk)
        gpos = np.array([gorder[(b, h)] for b, h in zip(blk, half)])
        o = np.lexsort((lidx, gpos))
        blk, rel, half, lidx, gpos = blk[o], rel[o], half[o], lidx[o], gpos[o]
        # position of each edge within its group (edges sorted by gpos so
        # same-group edges are contiguous)
        gid = blk * 2 + half
        order_in_g = np.zeros(n, np.int64)
        runs = np.flatnonzero(np.diff(gid) != 0)
        bounds = np.concatenate([[0], runs + 1, [n]])
        for bi in range(len(bounds) - 1):
            s, e = bounds[bi], bounds[bi + 1]
            order_in_g[s:e] = np.arange(e - s)
        su = np.array([sub_base[(b, h)] for b, h in zip(blk, half)]) + (order_in_g >> 7)
        epos = order_in_g & 127

        midx = np.zeros((NSUB, 128), np.int16)
        midx[su, epos] = lidx.astype(np.int16)
        # interleaved per-sub: cols [0:128) = S^T[e,d]; [128:256) = S_dmaj[d,e]
        sst = np.zeros((128, NSUB * 256), NP_F8)
        sst[epos, su * 256 + rel] = 1.0
        sst[rel, su * 256 + 128 + epos] = 1.0

        idxm = np.zeros((128, IDXC), np.int16)
        for ci, (h, k, s0) in enumerate(calls):
            flat = midx[s0:s0 + k].reshape(k * 128)
            w16 = flat.reshape(k * 8, 16).T
            idxm[:, call_off[ci]:call_off[ci] + k * 8] = np.tile(w16, (8, 1))

        xs = np.asarray(x[c * SH:(c + 1) * SH], np.float32)
        xT = np.zeros((128, SHP), NP_BF16)
        xT[:, permpos[c * SH:(c + 1) * SH]] = xs.T.astype(NP_BF16)

        m = {
            "xT": xT,
            "idxm": idxm,
            "sst": sst,
            "ident": np.eye(128, dtype=NP_BF16),
        }
        for li in range(3):
            W = np.asarray(Ws[li], np.float32)
            wf = np.concatenate(
                [W, _wvec(W, np.asarray(a_srcs[li], np.float32)),
                 _wvec(W, np.asarray(a_dsts[li], np.float32))], axis=1)  # [fin, 264]
            fin = wf.shape[0]
            kt = fin // 128
            m[f"wf{li + 1}"] = wf.reshape(kt, 128, 264).transpose(1, 0, 2).astype(
                NP_BF16).copy()
            m[f"b{li + 1}"] = np.tile(np.asarray(bs[li], np.float32)[None, :],
                                      (128, 1))
        in_maps.append(m)

    sched = dict(calls=calls, NSUB=NSUB, NCALLS=NCALLS, IDXC=IDXC,
                 call_off=call_off, sub_blk=sub_blk,
                 sub_start=sub_start, sub_stop=sub_stop, sub_first=sub_first,
                 permpos=permpos)
    return in_maps, sched


# --------------------------------------------------------------- bass kernel
def _build(sched, profile=False):
    calls = sched["calls"]
    NSUB, NCALLS, IDXC = sched["NSUB"], sched["NCALLS"], sched["IDXC"]
    call_off = sched["call_off"]
    sub_blk, sub_start, sub_stop, sub_first = (
        sched["sub_blk"], sched["sub_start"], sched["sub_stop"],
        sched["sub_first"])

    nc = bacc.Bacc("TRN2", target_bir_lowering=False, debug=False,
                   num_devices=(1 if profile else NC),
                   num_swdge_queues=4, dynamic_dma_scratch_size=32768)

    xT_d = nc.dram_tensor("xT", [128, SHP], BF16, kind="ExternalInput")
    idxm_d = nc.dram_tensor("idxm", [128, IDXC], I16, kind="ExternalInput")
    sst_d = nc.dram_tensor("sst", [128, NSUB * 256], FP8, kind="ExternalInput")
    ident_d = nc.dram_tensor("ident", [128, 128], BF16, kind="ExternalInput")
    wf_d = [nc.dram_tensor(f"wf{li + 1}", [128, (1 if li == 0 else 2), 264],
                           BF16, kind="ExternalInput") for li in range(3)]
    b_d = [nc.dram_tensor(f"b{li + 1}", [128, 256], F32, kind="ExternalInput")
           for li in range(3)]
    out_d = nc.dram_tensor("out", [SHP, 256], F32, kind="ExternalOutput")

    EXP = mybir.ActivationFunctionType.Exp
    RELU = mybir.ActivationFunctionType.Relu
    COPY = mybir.ActivationFunctionType.Copy

    with tile.TileContext(nc) as tc:
        with (
            tc.tile_pool(name="const", bufs=1) as const,
            tc.tile_pool(name="xt", bufs=1) as xtp,
            tc.tile_pool(name="loc", bufs=1) as locp,
            tc.tile_pool(name="tp", bufs=5) as tp,
            tc.tile_pool(name="sp", bufs=5) as sp,
            tc.tile_pool(name="gp", bufs=3) as gp,
            tc.tile_pool(name="ep", bufs=6) as ep,
            tc.tile_pool(name="fp", bufs=3) as fpool,
            tc.tile_pool(name="srp", bufs=4) as srp,
            tc.tile_pool(name="dn", bufs=3) as dn,
            tc.tile_pool(name="psb", bufs=3, space="PSUM") as psb,
            tc.tile_pool(name="psd", bufs=1, space="PSUM") as psd,
            tc.tile_pool(name="pst", bufs=2, space="PSUM") as pst,
            tc.tile_pool(name="pse", bufs=2, space="PSUM") as pse,
            tc.tile_pool(name="dram", bufs=1, space="DRAM") as dram,
        ):
            ident = const.tile([128, 128], BF16, tag="ident")
            nc.sync.dma_start(ident[:], ident_d[:])
            idxm = const.tile([128, IDXC], I16, tag="idxm")
            nc.sync.dma_start(idxm[:], idxm_d[:])
            fdata = locp.tile([128, NBLK, 8], BF16, tag="fdata")
            exs = locp.tile([128, NBLK, 4], F32, tag="exs")
            wf_sb = []
            b_sb = []
            for li in range(3):
                kt = 1 if li == 0 else 2
                w = const.tile([128, kt, 264], BF16, tag=f"wf{li}")
                nc.sync.dma_start(w[:], wf_d[li][:])
                wf_sb.append(w)
                bt = const.tile([128, 256], F32, tag=f"b{li}")
                nc.sync.dma_start(bt[:], b_d[li][:])
                b_sb.append(bt)

            XT = []
            for b in range(NBLK):
                xtb = xtp.tile([128, 2, 128], BF16, tag=f"XT{b}")
                XT.append(xtb)
                nc.sync.dma_start(xtb[:, 0, :], xT_d[:, b * 128:(b + 1) * 128])

            tables = []
            for li in range(3):
                t_sh = dram.tile([SHP, TBLB], FP8, tag=f"tsh{li}")
                t_full = dram.tile([NC * SHP, TBLB], FP8, tag=f"tfull{li}",
                                   addr_space="Shared")
                tables.append((t_sh, t_full))

            for li in range(3):
                kt = 1 if li == 0 else 2
                t_sh, t_full = tables[li]
                # ---------------- dense: h | E1 E2 | F1 F2 ------------------
                for t in range(NBLK):
                    pd = psd.tile([128, 264], F32, tag="pd")
                    for kk in range(kt):
                        nc.tensor.matmul(
                            pd[:], XT[t][:, kk, :],
                            wf_sb[li][:, kk, :],
                            start=(kk == 0), stop=(kk == kt - 1))
                    tb = dn.tile([128, TBLB], FP8, tag="tb")
                    nc.vector.tensor_copy(tb[:, 0:256], pd[:, 0:256])
                    tbb = tb[:].bitcast(BF16)  # [128, 256]
                    nc.scalar.activation(tbb[:, 128:132], pd[:, 256:260], EXP)
                    nc.scalar.activation(tbb[:, 132:136], pd[:, 256:260], EXP,
                                         scale=NEG)
                    nc.scalar.activation(fdata[:, t, 0:4], pd[:, 260:264], EXP)
                    nc.scalar.activation(fdata[:, t, 4:8], pd[:, 260:264], EXP,
                                         scale=NEG)
                    nc.sync.dma_start(t_sh[t * 128:(t + 1) * 128, :], tb[:])
                    # ex_self = max(E1*F1, E2*F2)
                    p1 = ep.tile([128, 4], F32, tag="p1s")
                    nc.vector.tensor_tensor(p1[:], tbb[:, 128:132],
                                            fdata[:, t, 0:4],
                                            mybir.AluOpType.mult)
                    p2 = ep.tile([128, 4], F32, tag="p2s")
                    nc.vector.tensor_tensor(p2[:], tbb[:, 132:136],
                                            fdata[:, t, 4:8],
                                            mybir.AluOpType.mult)
                    nc.vector.tensor_tensor(exs[:, t, :], p1[:], p2[:],
                                            mybir.AluOpType.max)
                # ---------------- all-gather the table ----------------------
                if profile:
                    nc.sync.dma_start(t_full[0:SHP, :], t_sh[:])
                else:
                    nc.gpsimd.collective_compute(
                        "AllGather", mybir.AluOpType.bypass,
                        ins=[t_sh[:]], outs=[t_full[:]],
                        replica_groups=[list(range(NC))])
                # ---------------- edge phase (sw-pipelined emission) ---------
                pcur = None
                live = {}
                selfrow = {}

                def emit_gather(ci):
                    h, k, s0 = calls[ci]
                    tt = tp.tile([128, CALL_K, TBLB], FP8, tag="tt")
                    nc.gpsimd.dma_gather(
                        tt[:, :k, :], t_full[h * HALF:(h + 1) * HALF, :],
                        idxm[:, call_off[ci]:call_off[ci] + k * 8],
                        k * 128, k * 128, TBLB, queue_num=ci % 4)
                    st = sp.tile([128, CALL_K * 256], FP8, tag="st")
                    nc.scalar.dma_start(
                        st[:, :k * 256], sst_d[:, s0 * 256:(s0 + k) * 256])
                    live[ci] = [tt, st]
                    for s in range(k):
                        su = s0 + s
                        if sub_first[su]:
                            b = int(sub_blk[su])
                            srt = srp.tile([128, TBLB], FP8, tag="selfrow")
                            nc.sync.dma_start(
                                srt[:], t_sh[b * 128:(b + 1) * 128, :])
                            selfrow[b] = srt

                def emit_score(ci):
                    h, k, s0 = calls[ci]
                    tt, st = live[ci]
                    pe_ps = pse.tile([128, CALL_K, 8], F32, tag="pe")
                    for s in range(k):
                        b = int(sub_blk[s0 + s])
                        sd = st[:, s * 256 + 128:s * 256 + 256]
                        nc.tensor.matmul(
                            pe_ps[:, s, :], sd, fdata[:, b, 0:8],
                            start=True, stop=True)
                    ttb = tt[:].bitcast(BF16)  # [128, CALL_K, 256]
                    p1 = ep.tile([128, CALL_K, 4], F32, tag="p1")
                    nc.vector.tensor_tensor(
                        p1[:, :k, :], ttb[:, :k, 128:132], pe_ps[:, :k, 0:4],
                        mybir.AluOpType.mult)
                    p2 = ep.tile([128, CALL_K, 4], F32, tag="p2")
                    nc.vector.tensor_tensor(
                        p2[:, :k, :], ttb[:, :k, 132:136], pe_ps[:, :k, 4:8],
                        mybir.AluOpType.mult)
                    g = gp.tile([128, CALL_K, 260], BF16, tag="g")
                    g4 = g[:, :k, :].rearrange("p s (h c) -> p s h c", h=HEADS)
                    nc.vector.tensor_tensor(
                        g4[:, :, :, 64], p1[:, :k, :], p2[:, :k, :],
                        mybir.AluOpType.max)
                    live[ci] += [pe_ps, g]

                def emit_agg(ci):
                    nonlocal pcur
                    h, k, s0 = calls[ci]
                    tt, st, pe_ps, g = live.pop(ci)
                    g4 = g[:, :k, :].rearrange("p s (h c) -> p s h c", h=HEADS)
                    exb = g4[:, :, :, 64].unsqueeze(3).broadcast_to(
                        [128, k, HEADS, C])
                    tt4 = tt[:, :k, 0:256].rearrange(
                        "p s (h c) -> p s h c", h=HEADS)
                    nc.vector.tensor_tensor(
                        g4[:, :, :, 0:64], tt4, exb, mybir.AluOpType.mult)
                    for s in range(k):
                        su = s0 + s
                        if sub_start[su]:
                            pcur = psb.tile([128, 260], F32, tag="pb")
                        nc.tensor.matmul(
                            pcur[:], st[:, s * 256:s * 256 + 128], g[:, s, :],
                            start=bool(sub_start[su]), stop=bool(sub_stop[su]))
                        if sub_stop[su]:
                            b = int(sub_blk[su])
                            ph = pcur[:].rearrange("p (h c) -> p h c", h=HEADS)
                            dn4 = ep.tile([128, 4], F32, tag="dn4")
                            nc.vector.tensor_tensor(
                                dn4[:], ph[:, :, 64], exs[:, b, :],
                                mybir.AluOpType.add)
                            rec = ep.tile([128, 4], F32, tag="rec")
                            nc.vector.reciprocal(rec[:], dn4[:])
                            srt = selfrow.pop(b)
                            sr4 = srt[:, 0:256].rearrange(
                                "p (h c) -> p h c", h=HEADS)
                            exsb = exs[:, b, :].unsqueeze(2).broadcast_to(
                                [128, HEADS, C])
                            selfnum = fpool.tile([128, 256], BF16, tag="sn")
                            sn4 = selfnum[:].rearrange(
                                "p (h c) -> p h c", h=HEADS)
                            nc.vector.tensor_tensor(
                                sn4[:], sr4, exsb, mybir.AluOpType.mult)
                            t1 = fpool.tile([128, 256], F32, tag="t1")
                            fin = fpool.tile([128, 256], F32, tag="fin")
                            for hh in range(HEADS):
                                nc.vector.scalar_tensor_tensor(
                                    t1[:, hh * 64:(hh + 1) * 64],
                                    selfnum[:, hh * 64:(hh + 1) * 64],
                                    rec[:, hh:hh + 1],
                                    b_sb[li][:, hh * 64:(hh + 1) * 64],
                                    mybir.AluOpType.mult, mybir.AluOpType.add)
                            for hh in range(HEADS):
                                nc.vector.scalar_tensor_tensor(
                                    fin[:, hh * 64:(hh + 1) * 64],
                                    pcur[:, hh * 65:hh * 65 + 64],
                                    rec[:, hh:hh + 1],
                                    t1[:, hh * 64:(hh + 1) * 64],
                                    mybir.AluOpType.mult, mybir.AluOpType.add)
                            if li < 2:
                                xr = fpool.tile([128, 256], BF16, tag="xr")
                                nc.scalar.activation(xr[:], fin[:], RELU)
                                for half in range(2):
                                    pt = pst.tile([128, 128], BF16, tag="pt")
                                    nc.tensor.transpose(
                                        pt[:],
                                        xr[:, half * 128:(half + 1) * 128],
                                        ident[:])
                                    nc.scalar.activation(
                                        XT[b][:, half, :], pt[:], COPY)
                            else:
                                nc.sync.dma_start(
                                    out_d[b * 128:(b + 1) * 128, :], fin[:])

                for ci in range(len(calls) + 3):
                    if ci < len(calls):
                        emit_gather(ci)
                    if ci >= 2 and ci - 2 < len(calls):
                        emit_score(ci - 2)
                    if ci >= 3:
                        emit_agg(ci - 3)
    nc.compile()
    return nc


def _get(x, edge_index, Ws, a_srcs, a_dsts, bs, profile=False):
    in_maps, sched = _prep(x, edge_index, Ws, a_srcs, a_dsts, bs)
    nc = _build(sched, profile=profile)
    return nc, in_maps, sched


def kernel(x, edge_index, W1, a_src1, a_dst1, b1, W2, a_src2, a_dst2, b2,
           W3, a_src3, a_dst3, b3):
    x = np.asarray(x)
    nc, in_maps, sched = _get(x, np.asarray(edge_index),
                              [W1, W2, W3], [a_src1, a_src2, a_src3],
                              [a_dst1, a_dst2, a_dst3], [b1, b2, b3])
    res = run_bass_kernel_spmd(nc, in_maps, core_ids=list(range(NC)))
    permpos = sched["permpos"]
    out = np.concatenate(
        [res.results[c]["out"][permpos[c * SH:(c + 1) * SH]] for c in range(NC)],
        axis=0)
    return out[:N].astype(np.float32)
